# revision 53
# baseline (speedup 1.0000x reference)
"""Bass/Trainium2 kernel for nn_DecoderAttn: batch-1 attention decoder step.

Sharding over 8 NeuronCores (tensor-parallel), reordered so all
skew-independent local work streams first and DMA never idles:

  stream order  = A(W_attn 8.4MB) -> W_hh(16.8) -> W_comb-x-rows(4.2)
                  -> E-cols(4.2) -> W_comb-aa-cols(4.2) -> W_ih(16.8)
                  -> W_out fp8(65.5)
  local work    = stage A matvec, all h@W_hh^T gate matmuls, x@W_comb-rows
                  (these need no collective result, so they consume the
                  stream while the first AllGather absorbs cross-core skew)

Collectives (4 total):
  AG-A   logits [1,512] f16  -> full logits everywhere (also feeds logZ)
  AR-li  lstm_in partials [1,4096] f32 AllReduce: W_comb is split by
         CONTRACTION column - core k sums over its own 512 input columns
         (x half early+local via a staged SBUF vector, aa half after
         AG-A) producing partials for all 4096 rows
  AG-h   h_new slice -> full h for the vocab projection
  AG-s   per-core sumexp -> global logsumexp

colsum(E-col-slice) is computed on the host during sharding (it exists
only because log_softmax is folded as aa = l@E - logZ*colsum; the
reference's own FLOPs all stay on device).

Stage E runs fp8-e4m3 DoubleRow, weight tiles streamed on the sync queue
(no head-of-line blocking behind ACT work), exp+accum read PSUM directly
and run per-window. The tail scatters word [1,SV]f16 -> [16,1000] via a
DRAM hop overlapped with AG-s, broadcasts -lse to 16 partitions with a
1-cycle matmul, subtracts in one 1000-elem DVE op, and DMAs out.

PSUM: two static rings — "pg" 4 banks (gate accumulators, open from the
early W_hh matmuls until the last W_ih matmul) and "po" 4 banks
(everything else: stage-A logits, lstm_in col-group partials, stage-E
window chunks, nlse broadcast).

Measured (core 0, trace): ~452-466us vs 535us baseline, rel err 7.5e-3
(gate 2e-2). Roofline: 120.4 MB/core of weights at the ~330 GB/s
all-cores-streaming HBM share = ~364us floor; the rest is launch skew
absorbed at the first collective (~45-90us run-to-run) plus residual
collective-straggler stalls. fp8 for the f16 chain stages was emulated
and FAILS the gate (3.9-6e-2), so chain bytes are irreducible.
"""

import sys

if '/opt/trn_rl_repo' not in sys.path:
    sys.path.insert(0, '/opt/trn_rl_repo')

import numpy as np
import ml_dtypes

import concourse.bass as bass
import concourse.bacc as bacc
import concourse.tile as tile
import concourse.mybir as mybir
from concourse.bass_utils import run_bass_kernel_spmd

F32 = mybir.dt.float32
F16 = mybir.dt.float16
F8 = mybir.dt.float8e4
E4M3 = ml_dtypes.float8_e4m3
DR = mybir.MatmulPerfMode.DoubleRow

H = 4096
L = 4096
V = 128000
NC = 8
SH = H // NC        # 512 hidden slice
SL = L // NC        # 512 logit slice
SV = V // NC        # 16000 vocab slice

NW = 8              # stage-E vocab windows per core
WV = SV // NW       # 2000 vocab per window
NCH = 4             # psum chunks per window
CV = WV // NCH      # 500 vocab per chunk
NJ = H // 256       # 16 DoubleRow contraction steps (256 each)

N_I = H // 128      # 32 contraction chunks for K=4096
N_I2 = 2 * H // 128  # 64 for K=8192

_compiled = {}


def _build(bias_zero, cbz):
    nc = bacc.Bacc("TRN2", target_bir_lowering=False, debug=False, num_devices=NC)

    # ---- kernel I/O (per-core shards, same names across cores) ----
    d_h0 = nc.dram_tensor("h0f", [H], F16, kind="ExternalInput")
    d_x0 = nc.dram_tensor("x0f", [H], F16, kind="ExternalInput")
    d_c0 = nc.dram_tensor("c0s", [SH], F32, kind="ExternalInput")
    d_ba = nc.dram_tensor("ba", [SL], F32, kind="ExternalInput")
    d_cs = nc.dram_tensor("cs", [SH], F32, kind="ExternalInput")
    d_bg = nc.dram_tensor("bg", [4 * SH], F32, kind="ExternalInput")
    d_bo = nc.dram_tensor("bo", [SV], F32, kind="ExternalInput")
    d_wa = nc.dram_tensor("wa", [8, 128, 8 * SL], F16, kind="ExternalInput")
    d_e = nc.dram_tensor("e", [4, 128, 8 * SH], F16, kind="ExternalInput")
    d_xo = nc.dram_tensor("xo", [SH], F16, kind="ExternalInput")
    d_wcx = nc.dram_tensor("wcx", [4, 128, H], F16, kind="ExternalInput")
    d_wca = nc.dram_tensor("wca", [4, 128, H], F16, kind="ExternalInput")
    d_whh = nc.dram_tensor("whh", [16, 128, 2 * 2048], F16, kind="ExternalInput")
    d_wih = nc.dram_tensor("wih", [16, 128, 2 * 2048], F16, kind="ExternalInput")
    d_wo = nc.dram_tensor("wo", [NW, NJ // 2, 128, 2, 2, WV], F8,
                          kind="ExternalInput")
    d_out = nc.dram_tensor("out", [1, SV], F16, kind="ExternalOutput")
    if not cbz:
        d_bcf = nc.dram_tensor("bcf", [H], F32, kind="ExternalInput")

    rg = [list(range(NC))]

    with tile.TileContext(nc) as tc:
        with (
            tc.tile_pool(name="singles", bufs=1) as sg,
            tc.tile_pool(name="cw", bufs=10) as cw,      # chain weight stream
            tc.tile_pool(name="wop", bufs=10) as wop,    # W_out fp8 stream
            tc.tile_pool(name="small", bufs=1) as sm,    # small working tiles
            tc.tile_pool(name="psum", bufs=1, space="PSUM") as ps,
            tc.tile_pool(name="dram", bufs=1, space="DRAM") as dr,
        ):
            def pacc(name):
                """General-purpose PSUM accumulator, 4-bank FIFO ring.
                [16,512] declares 16 partitions on the 2KB bank so [16,1]
                slices exist for the nlse broadcast matmul."""
                return ps.tile([16, 512], F32, tag="po", bufs=4, name=name)

            def pgacc(name):
                """Gate accumulators: 4 banks held from the early W_hh
                matmuls until the last W_ih matmul."""
                return ps.tile([16, 512], F32, tag="pg", bufs=4, name=name)

            # ---------- small loads ----------
            hx = sg.tile([128, 64], F16, tag="hx")       # [h; x], elem 64p+i
            nc.sync.dma_start(hx[0:64, :], d_h0[:].rearrange("(p i) -> p i", p=64))
            nc.sync.dma_start(hx[64:128, :], d_x0[:].rearrange("(p i) -> p i", p=64))
            ht = sg.tile([128, 32], F16, tag="ht")       # h, elem 32p+i
            nc.sync.dma_start(ht[:], d_h0[:].rearrange("(p i) -> p i", p=128))
            xo = sg.tile([128, 4], F16, tag="xo")        # own x slice, 128k+p
            nc.sync.dma_start(xo[:], d_xo[:].rearrange("(k p) -> p k", p=128))
            c0t = sg.tile([1, SH], F32, tag="c0t")
            nc.sync.dma_start(c0t[:], d_c0[:].rearrange("n -> () n"))
            bat = sg.tile([1, SL], F32, tag="bat")
            nc.sync.dma_start(bat[:], d_ba[:].rearrange("n -> () n"))
            cs_sb = sg.tile([1, SH], F32, tag="cs_sb")
            nc.sync.dma_start(cs_sb[:], d_cs[:].rearrange("n -> () n"))
            bgt = sg.tile([1, 4 * SH], F32, tag="bgt")
            nc.sync.dma_start(bgt[:], d_bg[:].rearrange("n -> () n"))

            # ---------- stage A: attn logits = [h;x] @ W_attn^T ----------
            pa_t = pacc("pa")
            pa = pa_t[0:1, :]
            for blk in range(8):
                wt = cw.tile([128, 8, SL], F16, tag="cw")
                nc.sync.dma_start(wt[:], d_wa[blk].rearrange("p (j n) -> p j n", j=8))
                for j in range(8):
                    i = 8 * blk + j
                    nc.tensor.matmul(pa, hx[:, i:i + 1],
                                     wt[:, j, :],
                                     start=(i == 0), stop=(i == N_I2 - 1))
            logits_loc = sm.tile([1, SL], F16, tag="vloc", bufs=2)
            nc.vector.tensor_add(logits_loc[:], pa, bat[:])
            ag_a_in = dr.tile([1, SL], F16, tag="agai")
            ag_a_out = dr.tile([NC, SL], F16, tag="agao")
            nc.gpsimd.dma_start(ag_a_in[:], logits_loc[:])
            nc.gpsimd.collective_compute(
                "AllGather", mybir.AluOpType.bypass,
                ins=[ag_a_in.opt()], outs=[ag_a_out.opt()], replica_groups=rg)

            # ---------- local-only D h-side: h @ W_hh^T (all 16 tiles) ----
            pg_t = [pgacc(f"pg{b}") for b in range(4)]
            pg = [t[0:1, :] for t in pg_t]
            for blk in range(16):
                wt = cw.tile([128, 2, 2048], F16, tag="cw")
                nc.sync.dma_start(wt[:], d_whh[blk].rearrange("p (j n) -> p j n", j=2))
                for j in range(2):
                    i = 2 * blk + j
                    for b in range(4):
                        nc.tensor.matmul(pg[b],
                                         ht[:, i:i + 1],
                                         wt[:, j, 512 * b:512 * (b + 1)],
                                         start=(i == 0), stop=False)

            # ---------- local-only C x-part (contraction-split) -----------
            # core k contributes sum over its own x slice: x_own @
            # W_comb[:, own x-cols] -> partials for ALL 4096 lstm_in rows.
            # Staged to SBUF so the 8 col-group banks recycle early.
            xacc = sg.tile([1, H], F16, tag="xacc")
            wcx_tiles = []
            for k in range(4):
                wt = cw.tile([128, H], F16, tag="cw", name=f"wcx_{k}")
                nc.sync.dma_start(wt[:], d_wcx[k])
                wcx_tiles.append(wt)
            for cg in range(8):
                bank_t = pacc(f"pxc{cg}")
                bank = bank_t[0:1, :]
                for k in range(4):
                    nc.tensor.matmul(bank, xo[:, k:k + 1],
                                     wcx_tiles[k][:, 512 * cg:512 * (cg + 1)],
                                     start=(k == 0), stop=(k == 3))
                nc.vector.tensor_copy(xacc[:, 512 * cg:512 * (cg + 1)], bank)

            # ---------- stage B: aa_own = l @ E_cols - logZ * colsum ------
            # logits are O(6): exp without max-subtraction is safe in f32.
            aw = sg.tile([128, 32], F16, tag="aw")       # raw logits
            nc.gpsimd.dma_start(
                aw[:],
                ag_a_out[:].rearrange("r n -> (r n)").rearrange("(p i) -> p i", p=128))
            ex = sm.tile([128, 32], F32, tag="ex")
            nc.scalar.activation(ex[:], aw[:], mybir.ActivationFunctionType.Exp)
            exs = sm.tile([128, 1], F32, tag="exs")
            nc.vector.tensor_reduce(exs[:], ex[:], mybir.AxisListType.X,
                                    mybir.AluOpType.add)
            s1 = sm.tile([1, 1], F32, tag="s1")
            nc.gpsimd.tensor_reduce(s1[:], exs[:], mybir.AxisListType.C,
                                    mybir.AluOpType.add)
            lnsb = sm.tile([1, 1], F32, tag="lnsb")
            nc.scalar.activation(lnsb[:], s1[:], mybir.ActivationFunctionType.Ln)
            nlz = sm.tile([1, 1], F32, tag="nlz")
            nc.vector.tensor_scalar_mul(nlz[:], lnsb[:], -1.0)
            pb_t = pacc("pb")
            pb = pb_t[0:1, :]
            for blk in range(4):
                et2 = cw.tile([128, 8, SH], F16, tag="cw")
                nc.sync.dma_start(et2[:], d_e[blk].rearrange("p (j n) -> p j n", j=8))
                for j in range(8):
                    i = 8 * blk + j
                    nc.tensor.matmul(pb, aw[:, i:i + 1],
                                     et2[:, j, :],
                                     start=(i == 0), stop=(i == N_I - 1))
            aa_loc = sm.tile([1, SH], F16, tag="vloc", bufs=2)
            nc.vector.scalar_tensor_tensor(aa_loc[:], cs_sb[:], nlz[:], pb,
                                           mybir.AluOpType.mult,
                                           mybir.AluOpType.add)
            # p-major relayout of aa_own via a DRAM hop (DVE is lane-locked)
            aa_scr = dr.tile([1, SH], F16, tag="aascr")
            nc.gpsimd.dma_start(aa_scr[:], aa_loc[:])
            av = sg.tile([128, 4], F16, tag="av")
            nc.gpsimd.dma_start(
                av[:], aa_scr[:].rearrange("() (k p) -> p k", p=128))

            # ---------- C aa-part + AllGather -> lstm_in ------------------
            # col-group cg accumulates aa_own @ W_comb[:, H+own aa-cols],
            # adds the staged x-part, and lands in an AllGather input.
            # AllGather + local 8-way DVE reduce beats AllReduce here:
            # measured AR(16KB f32) = 26us vs AG(8KB f16) = 5us, and the
            # local reduce on [128,32] tiles is ~8x32 cycles.
            ar_li_in = dr.tile([1, H], F16, tag="arli_i")
            ag_li_out = dr.tile([NC, H], F16, tag="arli_o")
            wca_tiles = []
            for k in range(4):
                wt = cw.tile([128, H], F16, tag="cw", name=f"wca_{k}")
                nc.sync.dma_start(wt[:], d_wca[k])
                wca_tiles.append(wt)
            if not cbz:
                bcf_sb = sg.tile([1, H], F32, tag="bcf")
                nc.sync.dma_start(bcf_sb[:], d_bcf[:].rearrange("n -> () n"))

            for cg in range(8):
                bank_t = pacc(f"paa{cg}")
                bank = bank_t[0:1, :]
                for k in range(4):
                    nc.tensor.matmul(bank, av[:, k:k + 1],
                                     wca_tiles[k][:, 512 * cg:512 * (cg + 1)],
                                     start=(k == 0), stop=(k == 3))
                li_stage = sm.tile([1, 512], F16, tag="listg", bufs=2)
                nc.vector.tensor_add(li_stage[:], bank,
                                     xacc[:, 512 * cg:512 * (cg + 1)])
                if not cbz:
                    nc.vector.tensor_add(li_stage[:], li_stage[:],
                                         bcf_sb[:, 512 * cg:512 * (cg + 1)])
                nc.gpsimd.dma_start(
                    ar_li_in[:, 512 * cg:512 * (cg + 1)], li_stage[:])
            nc.gpsimd.collective_compute(
                "AllGather", mybir.AluOpType.bypass,
                ins=[ar_li_in.opt()], outs=[ag_li_out.opt()], replica_groups=rg)

            # ---------- stage D: + lstm_in @ W_ih^T, gates, cell ----------
            # local 8-way reduce of the gathered partials, p-major layout
            lg = sg.tile([128, 8, 32], F16, tag="lg")
            nc.gpsimd.dma_start(
                lg[:], ag_li_out[:].rearrange("r (p i) -> p r i", p=128))
            li32 = sg.tile([128, 32], F32, tag="li32")
            nc.vector.tensor_add(li32[:], lg[:, 0, :], lg[:, 1, :])
            for r in range(2, NC):
                nc.vector.tensor_add(li32[:], li32[:], lg[:, r, :])
            li = sg.tile([128, 32], F16, tag="li")
            nc.vector.tensor_copy(li[:], li32[:])
            for blk in range(16):
                wt = cw.tile([128, 2, 2048], F16, tag="cw")
                nc.sync.dma_start(wt[:], d_wih[blk].rearrange("p (j n) -> p j n", j=2))
                for j in range(2):
                    i = 2 * blk + j
                    for b in range(4):
                        nc.tensor.matmul(pg[b],
                                         li[:, i:i + 1],
                                         wt[:, j, 512 * b:512 * (b + 1)],
                                         start=False, stop=(i == N_I - 1))
            s_i = sm.tile([1, SH], F32, tag="si")
            s_f = sm.tile([1, SH], F32, tag="sf")
            t_g = sm.tile([1, SH], F32, tag="tg")
            s_o = sm.tile([1, SH], F32, tag="so")
            Sg = mybir.ActivationFunctionType.Sigmoid
            Th = mybir.ActivationFunctionType.Tanh
            if cbz:
                # zero gate biases: activations read the PSUM directly
                nc.scalar.activation(s_i[:], pg[0], Sg)
                nc.scalar.activation(s_f[:], pg[1], Sg)
                nc.scalar.activation(s_o[:], pg[3], Sg)
                nc.scalar.activation(t_g[:], pg[2], Th)
            else:
                gsb = sm.tile([1, 4 * SH], F32, tag="gsb")
                for b in range(4):
                    nc.vector.tensor_add(gsb[:, 512 * b:512 * (b + 1)], pg[b],
                                         bgt[:, 512 * b:512 * (b + 1)])
                nc.scalar.activation(s_i[:], gsb[:, 0:SH], Sg)
                nc.scalar.activation(s_f[:], gsb[:, SH:2 * SH], Sg)
                nc.scalar.activation(s_o[:], gsb[:, 3 * SH:4 * SH], Sg)
                nc.scalar.activation(t_g[:], gsb[:, 2 * SH:3 * SH], Th)
            # cell math reuses the gate tiles in place:
            # s_f <- sig(f)*c0, s_i <- sig(i)*tanh(g), s_i <- c_new,
            # t_g <- tanh(c_new), hn <- sig(o)*tanh(c_new)
            nc.vector.tensor_mul(s_f[:], s_f[:], c0t[:])
            nc.vector.tensor_mul(s_i[:], s_i[:], t_g[:])
            nc.vector.tensor_add(s_i[:], s_i[:], s_f[:])
            nc.scalar.activation(t_g[:], s_i[:], Th)
            hn_loc = sm.tile([1, SH], F16, tag="vloc", bufs=2)
            nc.vector.tensor_mul(hn_loc[:], s_o[:], t_g[:])
            ag_h_in = dr.tile([1, SH], F16, tag="aghi")
            ag_h_out = dr.tile([NC, SH], F16, tag="agho")
            nc.gpsimd.dma_start(ag_h_in[:], hn_loc[:])
            nc.gpsimd.collective_compute(
                "AllGather", mybir.AluOpType.bypass,
                ins=[ag_h_in.opt()], outs=[ag_h_out.opt()], replica_groups=rg)

            # ---------- stage E prep: h as fp8 e4m3 ----------
            hn16 = sg.tile([128, 32], F16, tag="hn16")   # (p, i) = h[32p + i]
            nc.gpsimd.dma_start(
                hn16[:],
                ag_h_out[:].rearrange("r n -> (r n)").rearrange("(p i) -> p i", p=128))
            # h8[p, j, t, m]: e4m3(h[32p + 2j + t]); M padded to 16 so the
            # stationary kt step satisfies the dual-fp8 LDWEIGHTS
            # restriction (step % 16 == 0).
            h8 = sg.tile([128, NJ, 2, 16], F8, tag="h8")
            nc.vector.tensor_copy(
                h8[:, :, :, 0:1],
                hn16[:].rearrange("p (j t) -> p j t ()", t=2))

            # ---------- stage E: word = h @ W_out^T (fp8 DoubleRow) ----------
            # epilogue chunks stage through a small SBUF ring and stream to
            # a DRAM scratch via the VECTOR queue (no big SBUF word tile,
            # no head-of-line blocking of the sync-queue weight stream);
            # exp+accum reads the PSUM directly in parallel.
            wscr = dr.tile([1, SV], F16, tag="wscr")
            sums = sg.tile([1, NW * NCH], F32, tag="sums")

            for w in range(NW):
                po = [pacc(f"po_{w}_{c}") for c in range(NCH)]
                for jj in range(NJ // 2):
                    # two j-steps per tile: 8KB partition lines DMA
                    # faster than 4KB (measured ~310 vs ~330 GB/s); 16KB
                    # tiles measured no better and coarsen the ring.
                    wt = wop.tile([128, 2, 2, WV], F8, tag="wo")
                    nc.sync.dma_start(wt[:], d_wo[w, jj])
                    for j2 in range(2):
                        j = 2 * jj + j2
                        for c in range(NCH):
                            nc.tensor.matmul(po[c][0:1, 0:CV], h8[:, j, :, 0:1],
                                             wt[:, j2, :, CV * c:CV * (c + 1)],
                                             start=(j == 0), stop=(j == NJ - 1),
                                             perf_mode=DR)
                for c in range(NCH):
                    vabs = WV * w + CV * c
                    stg = sm.tile([1, 512], F16, tag="wstg", bufs=4)
                    wsl = stg[:, 0:CV]
                    if bias_zero:
                        nc.vector.tensor_copy(wsl, po[c][0:1, 0:CV])
                        esc = sm.tile([1, 512], F32, tag="esc", bufs=2)
                        nc.scalar.activation(
                            esc[:, 0:CV], po[c][0:1, 0:CV],
                            mybir.ActivationFunctionType.Exp,
                            accum_out=sums[:, NCH * w + c:NCH * w + c + 1])
                    else:
                        boc = sm.tile([1, 512], F32, tag="boc", bufs=4)
                        nc.scalar.dma_start(boc[:, 0:CV],
                                            d_bo[vabs:vabs + CV].rearrange("n -> () n"))
                        nc.vector.tensor_add(wsl, po[c][0:1, 0:CV], boc[:, 0:CV])
                        esc = sm.tile([1, 512], F32, tag="esc", bufs=2)
                        nc.scalar.activation(
                            esc[:, 0:CV], wsl,
                            mybir.ActivationFunctionType.Exp,
                            accum_out=sums[:, NCH * w + c:NCH * w + c + 1])
                    nc.gpsimd.dma_start(wscr[:, vabs:vabs + CV], wsl)

            # ---------- local sumexp -> global logsumexp ----------
            s_loc = sm.tile([1, 1], F32, tag="sloc")
            nc.vector.tensor_reduce(s_loc[:], sums[:], mybir.AxisListType.X,
                                    mybir.AluOpType.add)
            pack = sm.tile([1, 8], F32, tag="pack")
            nc.vector.memset(pack[:], 0.0)
            nc.vector.tensor_copy(pack[:, 0:1], s_loc[:])
            ag_s_in = dr.tile([1, 8], F32, tag="agsi")
            ag_s_out = dr.tile([NC, 8], F32, tag="agso")
            nc.gpsimd.dma_start(ag_s_in[:], pack[:])
            nc.gpsimd.collective_compute(
                "AllGather", mybir.AluOpType.bypass,
                ins=[ag_s_in.opt()], outs=[ag_s_out.opt()], replica_groups=rg)
            stat = sm.tile([1, NC], F32, tag="stat")
            nc.gpsimd.dma_start(stat[:], ag_s_out[:, 0:1].rearrange("r () -> () r"))
            gtot = sm.tile([1, 1], F32, tag="gtot")
            nc.vector.tensor_reduce(gtot[:], stat[:], mybir.AxisListType.X,
                                    mybir.AluOpType.add)
            lse = sm.tile([1, 1], F32, tag="lse")
            nc.scalar.activation(lse[:], gtot[:], mybir.ActivationFunctionType.Ln)
            nlse = sm.tile([1, 1], F32, tag="nlse")
            nc.vector.tensor_scalar_mul(nlse[:], lse[:], -1.0)

            # ---------- out = word - lse, partition-parallel ----------
            # wscr -> [16,1000] gather overlaps AG-s; nlse broadcasts to 16
            # partitions via a 1-cycle matmul; subtract is one 1000-elem
            # DVE op.
            word16 = sg.tile([16, 1000], F16, tag="word16")
            nc.sync.dma_start(word16[:],
                              wscr[:].rearrange("() (p n) -> p n", p=16))
            ones16 = sg.tile([1, 16], F32, tag="ones16")
            nc.vector.memset(ones16[:], 1.0)
            pnl = pacc("pnl")[0:16, 0:1]
            nc.tensor.matmul(pnl, ones16[:], nlse[:], start=True, stop=True)
            nc.vector.tensor_scalar(word16[:], word16[:], pnl, None,
                                    mybir.AluOpType.add)
            nc.sync.dma_start(d_out[:].rearrange("() (p n) -> p n", p=16),
                              word16[:])

    nc.compile()
    return nc


def _get_nc(bias_zero, cbz):
    key = (bias_zero, cbz)
    if key not in _compiled:
        _compiled[key] = _build(bias_zero, cbz)
    return _compiled[key]


def _shard_inputs(encoder_outputs, h0, c0, x0, W_attn, b_attn, W_comb, b_comb,
                  W_ih, b_ih, W_hh, b_hh, W_out, b_out):
    f = lambda a: np.ascontiguousarray(np.asarray(a), dtype=np.float32)
    E = f(encoder_outputs); W_attn = f(W_attn); W_comb = f(W_comb)
    W_ih = f(W_ih); W_hh = f(W_hh); W_out = f(W_out)
    h0f = f(h0).reshape(H).astype(np.float16)
    x0f = f(x0).reshape(H).astype(np.float16)
    c0f = f(c0).reshape(H)
    b_attn = f(b_attn); b_comb = f(b_comb); b_out = f(b_out)
    bg_full = f(b_ih) + f(b_hh)
    cbz = bool(np.all(b_comb == 0)) and bool(np.all(bg_full == 0))

    # E chunks: [blk, p, j, n] = E[32p + 8blk + j, hh0+n]
    E_r = E.reshape(128, 32, H)

    in_maps = []
    for k in range(NC):
        l0, hh0, v0 = k * SL, k * SH, k * SV
        wa = W_attn[l0:l0 + SL].T.reshape(128, 8, 8, SL) \
            .transpose(1, 0, 2, 3).reshape(8, 128, 8 * SL)
        e = E_r[:, :, hh0:hh0 + SH].reshape(128, 4, 8, SH) \
            .transpose(1, 0, 2, 3).reshape(4, 128, 8 * SH)
        cs = E[:, hh0:hh0 + SH].sum(axis=0)
        # contraction-split W_comb: core k sums over its own 512 of the
        # 8192 input columns, producing partials for all 4096 rows.
        # wcx[k][p, n] = W_comb[n, hh0 + 128k + p]       (x half)
        # wca[k][p, n] = W_comb[n, H + hh0 + 128k + p]   (aa half)
        wcx = np.ascontiguousarray(
            W_comb[:, hh0:hh0 + SH].T.reshape(4, 128, H))
        wca = np.ascontiguousarray(
            W_comb[:, H + hh0:H + hh0 + SH].T.reshape(4, 128, H))
        rows = np.concatenate([np.arange(g * H + hh0, g * H + hh0 + SH)
                               for g in range(4)])
        wih = W_ih[rows].T.reshape(128, 16, 2, 2048) \
            .transpose(1, 0, 2, 3).reshape(16, 128, 4096)
        whh = W_hh[rows].T.reshape(128, 16, 2, 2048) \
            .transpose(1, 0, 2, 3).reshape(16, 128, 4096)
        # wo[w, jj, p, j2, t, n] = W_out[v0 + WV*w + n, 32p + 4jj + 2j2 + t]
        G = W_out[v0:v0 + SV].T                      # [H, SV], H = 32p+2j+t
        wo = np.ascontiguousarray(
            G.reshape(128, NJ // 2, 2, 2, NW, WV).transpose(4, 1, 0, 2, 3, 5)
        ).astype(E4M3)
        im = {
            "h0f": h0f, "x0f": x0f, "c0s": np.ascontiguousarray(c0f[hh0:hh0 + SH]),
            "xo": np.ascontiguousarray(x0f[hh0:hh0 + SH]),
            "ba": np.ascontiguousarray(b_attn[l0:l0 + SL]),
            "cs": np.ascontiguousarray(cs, np.float32),
            "bg": np.ascontiguousarray(bg_full[rows]),
            "bo": np.ascontiguousarray(b_out[v0:v0 + SV]),
            "wa": np.ascontiguousarray(wa, np.float16),
            "e": np.ascontiguousarray(e, np.float16),
            "wcx": np.ascontiguousarray(wcx, np.float16),
            "wca": np.ascontiguousarray(wca, np.float16),
            "wih": np.ascontiguousarray(wih, np.float16),
            "whh": np.ascontiguousarray(whh, np.float16),
            "wo": wo,
        }
        if not cbz:
            im["bcf"] = np.ascontiguousarray(b_comb / NC, np.float32)
        in_maps.append(im)
    return in_maps, cbz


def _run(in_maps, bias_zero, cbz, trace=False):
    nc = _get_nc(bias_zero, cbz)
    return run_bass_kernel_spmd(nc, in_maps, list(range(NC)), trace=trace)


def _bias_zero(inputs):
    return bool(np.all(np.asarray(inputs["b_out"]) == 0))


def _gather(res):
    return np.concatenate([res.results[k]["out"] for k in range(NC)],
                          axis=1).astype(np.float32)


def kernel(**inputs):
    in_maps, cbz = _shard_inputs(**inputs)
    res = _run(in_maps, _bias_zero(inputs), cbz)
    return _gather(res)


def run_traced(**inputs):
    """test-only helper: returns (output, BassKernelResults with profiling)."""
    in_maps, cbz = _shard_inputs(**inputs)
    res = _run(in_maps, _bias_zero(inputs), cbz, trace=True)
    return _gather(res), res
